# revision 7
# baseline (speedup 1.0000x reference)
# Bayesian dense layer: y = x @ (w_loc + softplus(w_std) * eps_w) + (b_loc + softplus(b_std) * eps_b)
#   x: [8192, 4096] f32, w_*: [4096, 4096] f32, b_*: [1, 4096] f32 -> y: [8192, 4096] f32
#
# 8 cores in a 2 (batch) x 4 (d_out) grid; core c owns
#   y[(c//4)*4096 : +4096, (c%4)*1024 : +1024].
#
# Shipped kernel (build_bass_kv5, VARIANT="kv5"): K-outer with a bf16 SBUF
# accumulator. Host pre-packs all inputs partition-major (xtt[p, w, kt, c],
# w*t[p, kt, n]) so every DMA is 128 single-run descriptors. W streams in
# blocks of SCHED=(8,8,8,8) k-tiles (HW A/B: beats (4,4,8,8,8) by 3-8 us —
# 32 fewer DVE evacuations outweigh the longer first-block fill), computed as
# wl + ln(1+exp(ws))*we from bf16 params (one ACT table set via
# _patch_act_tables - no Exp/Ln table reloads); each 2-k-tile chunk is its
# own SBUF tile so matmuls only wait for their chunk. Each block sweeps the
# 32 batch strips with bf16 matmuls (full rate, fp32 PSUM), DVE-accumulated
# into the bf16 yacc; block 0 runs chunk-major across the first 4 strips to
# keep the in-order PE queue fed during the fill, bias is folded into block
# 0's evacuation, and the last strip evacuates per 512-col half to shorten
# the drain. Measured on HW (repeat-slope, R=257, interleaved dispatches):
# 552.5-555.8 us/iter (SCHED=(8,8,8,8)), fastest in every in-run A/B
# (f32r/b16 baselines ~571-587 us; graded baseline 608.9 us); rel err
# 3.798e-3. Floor analysis: 2048 512-col bf16 matmuls cost ~525 us at the
# observed ~2.0 GHz effective PE clock + ~13 us visible LDWEIGHTS; a bare
# LDW+MM probe of the same stream measures 551 us, so staging, softplus,
# accumulation and output traffic are fully hidden behind the PE.
import numpy as np

import concourse.bass as bass
from concourse import bacc
import concourse.mybir as mybir
import concourse.tile as tile
from concourse.bass_utils import run_bass_kernel_spmd

P = 128
BATCH, D_IN, D_OUT = 8192, 4096, 4096
B_SHARD, D_SHARD = 2, 4
M = BATCH // B_SHARD          # 4096 batch rows per core
N = D_OUT // D_SHARD          # 1024 output cols per core
K = D_IN                      # 4096 contraction
KT = K // P                   # 32 k-tiles
MT = M // P                   # 32 m-tiles
NMM = 512                     # matmul moving free dim (fp32 max)
G = 2                         # k-tiles per W-prep group (1MB DMAs)

F32 = mybir.dt.float32
F32R = mybir.dt.float32r
ACT = mybir.ActivationFunctionType

_CACHE = {}


def _patch_act_tables():
    """Make `natural_log_exp_and_others` the only ACT table set offering Exp
    and Ln, so the table-load pass hoists ONE LoadActFuncSet instead of
    reloading between every Exp and Ln (~1.3us per reload on the ACT engine).
    Set order (and so act_func_set_id indices) is preserved; we only remove
    Exp/Ln from the other sets in the copy handed to the pass."""
    import functools
    import concourse.bacc as _bacc_mod
    if getattr(_bacc_mod, "_act_tables_patched", False):
        return
    orig = _bacc_mod.get_activation_tables

    @functools.cache
    def patched(arch):
        tabs = {k: set(v) for k, v in orig(arch).items()}
        keep = "natural_log_exp_and_others"
        if keep in tabs:
            for name, funcs in tabs.items():
                if name != keep:
                    funcs.discard(ACT.Exp)
                    funcs.discard(ACT.Ln)
        return tabs

    _bacc_mod.get_activation_tables = patched
    _bacc_mod._act_tables_patched = True


def _declare_io(nc, M=M, N=N, K=K):
    xt = nc.dram_tensor("xt", [K, M], F32R, kind="ExternalInput").ap()
    wl = nc.dram_tensor("wl", [K, N], F32R, kind="ExternalInput").ap()
    ws = nc.dram_tensor("ws", [K, N], F32, kind="ExternalInput").ap()
    we = nc.dram_tensor("we", [K, N], F32, kind="ExternalInput").ap()
    bl = nc.dram_tensor("bl", [1, N], F32, kind="ExternalInput").ap()
    bs = nc.dram_tensor("bs", [1, N], F32, kind="ExternalInput").ap()
    be = nc.dram_tensor("be", [1, N], F32, kind="ExternalInput").ap()
    y = nc.dram_tensor("y", [M, N], F32, kind="ExternalOutput").ap()

    xt_r = xt.rearrange("(kt p) m -> p kt m", p=P)   # [128, KT, M]
    wl_r = wl.rearrange("(kt p) n -> p kt n", p=P)   # [128, KT, N]
    ws_r = ws.rearrange("(kt p) n -> p kt n", p=P)
    we_r = we.rearrange("(kt p) n -> p kt n", p=P)
    return xt_r, wl_r, ws_r, we_r, bl, bs, be, y


def _bias_bcast(nc, tc, const_pool, bl, bs, be, N=N):
    """b = bl + softplus(bs) * be broadcast to [128, N] in SBUF."""
    b_bcast = const_pool.tile([P, N], F32, name="b_bcast")
    with tc.tile_pool(name="bias_stage", bufs=1) as bias_pool:
        bl_t = bias_pool.tile([1, N], F32, name="bl_t")
        bs_t = bias_pool.tile([1, N], F32, name="bs_t")
        be_t = bias_pool.tile([1, N], F32, name="be_t")
        nc.scalar.dma_start(bl_t[:, :], bl[:, :])
        nc.scalar.dma_start(bs_t[:, :], bs[:, :])
        nc.scalar.dma_start(be_t[:, :], be[:, :])
        nc.scalar.activation(bs_t[:, :], bs_t[:, :], ACT.Exp)
        nc.scalar.activation(bs_t[:, :], bs_t[:, :], ACT.Ln, bias=1.0)
        nc.vector.tensor_mul(bs_t[:, :], bs_t[:, :], be_t[:, :])
        nc.vector.tensor_add(bl_t[:, :], bl_t[:, :], bs_t[:, :])
        nc.gpsimd.partition_broadcast(b_bcast[:, :], bl_t[:, :])
    return b_bcast


def build_bass(M=M, N=N, K=K, G=G, num_devices=8, repeat=1):
    KT, MT = K // P, M // P
    nc = bacc.Bacc(trn_type="TRN2", target_bir_lowering=False, debug=False,
                   num_devices=num_devices)
    xt_r, wl_r, ws_r, we_r, bl, bs, be, y = _declare_io(nc, M, N, K)

    from contextlib import ExitStack
    with tile.TileContext(nc) as tc, ExitStack() as rep_ctx:
        with tc.tile_pool(name="const", bufs=1) as const_pool:
            b_bcast = _bias_bcast(nc, tc, const_pool, bl, bs, be, N)

            # ---- W resident in SBUF: wres[p, kt, n] = wl + softplus(ws) * we
            with tc.tile_pool(name="wres_pool", bufs=1) as wres_pool, \
                 tc.tile_pool(name="wstage", bufs=2) as wstage_pool:
                if repeat > 1:
                    rep_ctx.enter_context(tc.For_i(0, repeat, 1))
                wres = wres_pool.tile([P, KT, N], F32R, name="wres")
                for kg in range(KT // G):
                    ks = kg * G
                    sp_t = wstage_pool.tile([P, G, N], F32, name="sp_t")
                    ep_t = wstage_pool.tile([P, G, N], F32, name="ep_t")
                    nc.sync.dma_start(sp_t[:], ws_r[:, ks:ks + G, :])
                    nc.sync.dma_start(ep_t[:], we_r[:, ks:ks + G, :])
                    nc.sync.dma_start(wres[:, ks:ks + G, :], wl_r[:, ks:ks + G, :])
                    nc.scalar.activation(sp_t[:], sp_t[:], ACT.Exp)
                    nc.scalar.activation(sp_t[:], sp_t[:], ACT.Ln, bias=1.0)
                    nc.vector.tensor_mul(sp_t[:], sp_t[:], ep_t[:])
                    nc.vector.tensor_add(wres[:, ks:ks + G, :],
                                         wres[:, ks:ks + G, :], sp_t[:])

                # ---- main loop: per 128-row batch strip, 32 fp32r matmuls per n-half
                with tc.tile_pool(name="xs_pool", bufs=2) as xs_pool, \
                     tc.tile_pool(name="psum_pool", bufs=3, space="PSUM") as psum_pool, \
                     tc.tile_pool(name="out_pool", bufs=2) as out_pool:
                    for m in range(MT):
                        xs = xs_pool.tile([P, KT, P], F32R, name="xs")
                        nc.scalar.dma_start(xs[:], xt_r[:, :, m * P:(m + 1) * P])
                        ps = psum_pool.tile([P, N], F32, name="ps")
                        for k in range(KT):
                            lhsT = xs[:, k, :]
                            for n in range(N // NMM):
                                nc.tensor.matmul(
                                    ps[:, n * NMM:(n + 1) * NMM],
                                    lhsT=lhsT,
                                    rhs=wres[:, k, n * NMM:(n + 1) * NMM],
                                    start=(k == 0),
                                    stop=(k == KT - 1),
                                )
                        outt = out_pool.tile([P, N], F32, name="outt")
                        nc.vector.tensor_add(outt[:], ps[:], b_bcast[:])
                        nc.sync.dma_start(y[m * P:(m + 1) * P, :], outt[:])
    nc.compile()
    return nc


def build_bass_kouter(KG=4, MG=4, M=M, N=N, K=K, num_devices=8, repeat=1,
                      xs_bufs=3):
    """K-outer order with an SBUF fp32 accumulator for the whole [M, N] output.

    W streams in KG-k-tile blocks spread evenly across the run (no big upfront
    fill stall); each block sweeps all 32 m-strips, accumulating psum into yacc.
    """
    KT, MT = K // P, M // P
    KB = KT // KG
    nc = bacc.Bacc(trn_type="TRN2", target_bir_lowering=False, debug=False,
                   num_devices=num_devices)
    xt_r, wl_r, ws_r, we_r, bl, bs, be, y = _declare_io(nc, M, N, K)

    from contextlib import ExitStack
    with tile.TileContext(nc) as tc, ExitStack() as rep_ctx:
        with tc.tile_pool(name="const", bufs=1) as const_pool:
            b_bcast = _bias_bcast(nc, tc, const_pool, bl, bs, be, N)

            with tc.tile_pool(name="yacc_pool", bufs=1) as yacc_pool, \
                 tc.tile_pool(name="wwin_pool", bufs=2) as wwin_pool, \
                 tc.tile_pool(name="wstage", bufs=1) as wstage_pool, \
                 tc.tile_pool(name="xs_pool", bufs=xs_bufs) as xs_pool, \
                 tc.tile_pool(name="psum_pool", bufs=4, space="PSUM") as psum_pool:
                if repeat > 1:
                    rep_ctx.enter_context(tc.For_i(0, repeat, 1))
                yacc = yacc_pool.tile([P, MT, N], F32, name="yacc")  # 128KB/part

                for kb in range(KB):
                    k0 = kb * KG
                    # W block: wwin[p, kj, n] = wl + softplus(ws)*we for k0..k0+KG
                    wwin = wwin_pool.tile([P, KG, N], F32R, name="wwin")
                    nc.sync.dma_start(wwin[:], wl_r[:, k0:k0 + KG, :])
                    for h in range(KG // 2):  # stage in 2-k-tile (1MB) chunks
                        hs = h * 2
                        sp_t = wstage_pool.tile([P, 2, N], F32, name="sp_t")
                        ep_t = wstage_pool.tile([P, 2, N], F32, name="ep_t")
                        nc.sync.dma_start(sp_t[:], ws_r[:, k0 + hs:k0 + hs + 2, :])
                        nc.sync.dma_start(ep_t[:], we_r[:, k0 + hs:k0 + hs + 2, :])
                        nc.scalar.activation(sp_t[:], sp_t[:], ACT.Exp)
                        nc.scalar.activation(sp_t[:], sp_t[:], ACT.Ln, bias=1.0)
                        nc.vector.tensor_mul(sp_t[:], sp_t[:], ep_t[:])
                        nc.vector.tensor_add(wwin[:, hs:hs + 2, :],
                                             wwin[:, hs:hs + 2, :], sp_t[:])

                    for mg in range(MT // MG):
                        m0 = mg * MG
                        xs = xs_pool.tile([P, KG, MG * P], F32R, name="xs")
                        nc.scalar.dma_start(
                            xs[:], xt_r[:, k0:k0 + KG, m0 * P:(m0 + MG) * P])
                        for mi in range(MG):
                            m = m0 + mi
                            ps = psum_pool.tile([P, N], F32, name="ps")
                            for kj in range(KG):
                                lhsT = xs[:, kj, mi * P:(mi + 1) * P]
                                for n in range(N // NMM):
                                    nc.tensor.matmul(
                                        ps[:, n * NMM:(n + 1) * NMM],
                                        lhsT=lhsT,
                                        rhs=wwin[:, kj,
                                                 n * NMM:(n + 1) * NMM],
                                        start=(kj == 0),
                                        stop=(kj == KG - 1),
                                    )
                            if kb == 0:
                                nc.vector.scalar_tensor_tensor(
                                    yacc[:, m, :], ps[:], 0.0, b_bcast[:],
                                    op0=mybir.AluOpType.add,
                                    op1=mybir.AluOpType.add)
                            else:
                                nc.vector.tensor_add(yacc[:, m, :],
                                                     yacc[:, m, :], ps[:])
                            if kb == KB - 1:
                                nc.sync.dma_start(y[m * P:(m + 1) * P, :],
                                                  yacc[:, m, :])
    nc.compile()
    return nc


BF16 = mybir.dt.bfloat16


def build_bass_kouter_b16(KG=8, MWIN=512, M=M, N=N, K=K, num_devices=8, repeat=1,
                          no_mm=False, no_evac=False, xs_bufs=3):
    """K-outer + SBUF fp32 accumulator, with x and W params staged as bf16.

    Halves DMA volume (x 33.5MB, W params 37.8MB per core); matmuls run bf16
    with fp32 PSUM accumulation. W is still computed on device from
    (w_loc, softplus(w_std), eps_w); softplus intermediate kept in fp32.
    """
    KT, MT = K // P, M // P
    KB = KT // KG
    MGT = MWIN // P                    # m-tiles per x window
    nc = bacc.Bacc(trn_type="TRN2", target_bir_lowering=False, debug=False,
                   num_devices=num_devices)
    xt = nc.dram_tensor("xt", [K, M], BF16, kind="ExternalInput").ap()
    wl = nc.dram_tensor("wl", [K, N], BF16, kind="ExternalInput").ap()
    ws = nc.dram_tensor("ws", [K, N], BF16, kind="ExternalInput").ap()
    we = nc.dram_tensor("we", [K, N], BF16, kind="ExternalInput").ap()
    bl = nc.dram_tensor("bl", [1, N], F32, kind="ExternalInput").ap()
    bs = nc.dram_tensor("bs", [1, N], F32, kind="ExternalInput").ap()
    be = nc.dram_tensor("be", [1, N], F32, kind="ExternalInput").ap()
    y = nc.dram_tensor("y", [M, N], F32, kind="ExternalOutput").ap()
    xt_r = xt.rearrange("(kt p) m -> p kt m", p=P)
    wl_r = wl.rearrange("(kt p) n -> p kt n", p=P)
    ws_r = ws.rearrange("(kt p) n -> p kt n", p=P)
    we_r = we.rearrange("(kt p) n -> p kt n", p=P)

    from contextlib import ExitStack
    with tile.TileContext(nc) as tc, ExitStack() as rep_ctx:
        with tc.tile_pool(name="const", bufs=1) as const_pool:
            b_bcast = _bias_bcast(nc, tc, const_pool, bl, bs, be, N)

            with tc.tile_pool(name="yacc_pool", bufs=1) as yacc_pool, \
                 tc.tile_pool(name="wwin_pool", bufs=2) as wwin_pool, \
                 tc.tile_pool(name="wstage", bufs=1) as wstage_pool, \
                 tc.tile_pool(name="xs_pool", bufs=xs_bufs) as xs_pool, \
                 tc.tile_pool(name="psum_pool", bufs=4, space="PSUM") as psum_pool:
                if repeat > 1:
                    rep_ctx.enter_context(tc.For_i(0, repeat, 1))
                yacc = yacc_pool.tile([P, MT, N], F32, name="yacc")  # 128KB/part

                for kb in range(KB):
                    k0 = kb * KG
                    wwin = wwin_pool.tile([P, KG, N], BF16, name="wwin")
                    nc.sync.dma_start(wwin[:], wl_r[:, k0:k0 + KG, :])
                    for h in range(KG // 2):  # 2-k-tile staging chunks
                        hs = h * 2
                        wsb_t = wstage_pool.tile([P, 2, N], BF16, name="wsb_t")
                        web_t = wstage_pool.tile([P, 2, N], BF16, name="web_t")
                        spf_t = wstage_pool.tile([P, 2, N], F32, name="spf_t")
                        nc.sync.dma_start(wsb_t[:], ws_r[:, k0 + hs:k0 + hs + 2, :])
                        nc.sync.dma_start(web_t[:], we_r[:, k0 + hs:k0 + hs + 2, :])
                        nc.scalar.activation(spf_t[:], wsb_t[:], ACT.Exp)
                        nc.scalar.activation(spf_t[:], spf_t[:], ACT.Ln, bias=1.0)
                        nc.vector.tensor_mul(spf_t[:], spf_t[:], web_t[:])
                        nc.vector.tensor_add(wwin[:, hs:hs + 2, :],
                                             wwin[:, hs:hs + 2, :], spf_t[:])

                    for mg in range(MT // MGT):
                        m0 = mg * MGT
                        xs = xs_pool.tile([P, KG, MWIN], BF16, name="xs")
                        nc.scalar.dma_start(
                            xs[:], xt_r[:, k0:k0 + KG, m0 * P:m0 * P + MWIN])
                        for mi in range(MGT):
                            m = m0 + mi
                            ps = psum_pool.tile([P, N], F32, name="ps")
                            if not no_mm:
                                for kj in range(KG):
                                    lhsT = xs[:, kj, mi * P:(mi + 1) * P]
                                    for n in range(N // NMM):
                                        nc.tensor.matmul(
                                            ps[:, n * NMM:(n + 1) * NMM],
                                            lhsT=lhsT,
                                            rhs=wwin[:, kj, n * NMM:(n + 1) * NMM],
                                            start=(kj == 0),
                                            stop=(kj == KG - 1),
                                        )
                            else:
                                nc.tensor.matmul(
                                    ps[:, 0:NMM], lhsT=xs[:, 0, mi * P:(mi + 1) * P],
                                    rhs=wwin[:, 0, 0:NMM], start=True, stop=True)
                            if no_evac:
                                if kb == KB - 1:
                                    nc.sync.dma_start(y[m * P:(m + 1) * P, :],
                                                      b_bcast[:].broadcast_to((P, N))
                                                      if False else b_bcast[:])
                                continue
                            if kb == 0:
                                nc.vector.scalar_tensor_tensor(
                                    yacc[:, m, :], ps[:], 0.0, b_bcast[:],
                                    op0=mybir.AluOpType.add,
                                    op1=mybir.AluOpType.add)
                            else:
                                nc.vector.tensor_add(yacc[:, m, :],
                                                     yacc[:, m, :], ps[:])
                            if kb == KB - 1:
                                nc.sync.dma_start(y[m * P:(m + 1) * P, :],
                                                  yacc[:, m, :])
    nc.compile()
    return nc


def build_bass_mres(CH=2, SG=4, XW=512, M=M, N=N, K=K, num_devices=8, repeat=1,
                    xs_bufs=2, psum_bufs=4, XCH=8):
    """W fully resident in SBUF as bf16 (64KB/partition); m-outer strips with
    full-K PSUM accumulation — no SBUF accumulator, one evacuation per strip.

    W stages in CH-k-tile chunks (DMA + Exp/Ln softplus + DVE mul/add); while
    the fill runs, the first SG-strip group's matmuls consume chunks as they
    land, so the PE never waits long. Strips then stream PE-bound; each strip
    does 32 accumulating matmuls per n-half in PSUM, one DVE bias-fold to an
    f32 out tile, and a DMA to y.
    """
    KT, MT = K // P, M // P
    nc = bacc.Bacc(trn_type="TRN2", target_bir_lowering=False, debug=False,
                   num_devices=num_devices)
    xt = nc.dram_tensor("xt", [K, M], BF16, kind="ExternalInput").ap()
    wl = nc.dram_tensor("wl", [K, N], BF16, kind="ExternalInput").ap()
    ws = nc.dram_tensor("ws", [K, N], BF16, kind="ExternalInput").ap()
    we = nc.dram_tensor("we", [K, N], BF16, kind="ExternalInput").ap()
    bl = nc.dram_tensor("bl", [1, N], F32, kind="ExternalInput").ap()
    bs = nc.dram_tensor("bs", [1, N], F32, kind="ExternalInput").ap()
    be = nc.dram_tensor("be", [1, N], F32, kind="ExternalInput").ap()
    y = nc.dram_tensor("y", [M, N], F32, kind="ExternalOutput").ap()
    xt_r = xt.rearrange("(kt p) m -> p kt m", p=P)
    wl_r = wl.rearrange("(kt p) n -> p kt n", p=P)
    ws_r = ws.rearrange("(kt p) n -> p kt n", p=P)
    we_r = we.rearrange("(kt p) n -> p kt n", p=P)

    from contextlib import ExitStack
    with tile.TileContext(nc) as tc, ExitStack() as rep_ctx:
        with tc.tile_pool(name="const", bufs=1) as const_pool:
            b_bcast = _bias_bcast(nc, tc, const_pool, bl, bs, be, N)

            with tc.tile_pool(name="wres_pool", bufs=1) as wres_pool, \
                 tc.tile_pool(name="wstage", bufs=2) as wstage_pool, \
                 tc.tile_pool(name="xs_pool", bufs=xs_bufs) as xs_pool, \
                 tc.tile_pool(name="out_pool", bufs=3) as out_pool, \
                 tc.tile_pool(name="psum_pool", bufs=psum_bufs,
                              space="PSUM") as psum_pool:
                if repeat > 1:
                    rep_ctx.enter_context(tc.For_i(0, repeat, 1))
                wres = wres_pool.tile([P, KT, N], BF16, name="wres")
                for kg in range(KT // CH):
                    ks = kg * CH
                    wsb_t = wstage_pool.tile([P, CH, N], BF16, name="wsb_t")
                    web_t = wstage_pool.tile([P, CH, N], BF16, name="web_t")
                    spf_t = wstage_pool.tile([P, CH, N], F32, name="spf_t")
                    nc.sync.dma_start(wres[:, ks:ks + CH, :],
                                      wl_r[:, ks:ks + CH, :])
                    nc.sync.dma_start(wsb_t[:], ws_r[:, ks:ks + CH, :])
                    nc.sync.dma_start(web_t[:], we_r[:, ks:ks + CH, :])
                    nc.scalar.activation(spf_t[:], wsb_t[:], ACT.Exp)
                    nc.scalar.activation(spf_t[:], spf_t[:], ACT.Ln, bias=1.0)
                    nc.vector.tensor_mul(spf_t[:], spf_t[:], web_t[:])
                    nc.vector.tensor_add(wres[:, ks:ks + CH, :],
                                         wres[:, ks:ks + CH, :], spf_t[:])

                for sg in range(MT // SG):
                    m0 = sg * SG
                    xs = xs_pool.tile([P, KT, XW], BF16, name="xs")
                    for xc in range(KT // XCH):
                        nc.scalar.dma_start(
                            xs[:, xc * XCH:(xc + 1) * XCH, :],
                            xt_r[:, xc * XCH:(xc + 1) * XCH,
                                 m0 * P:m0 * P + XW])
                    for si in range(SG):
                        m = m0 + si
                        ps = psum_pool.tile([P, N], F32, name="ps")
                        for kj in range(KT):
                            lhsT = xs[:, kj, si * P:(si + 1) * P]
                            for n in range(N // NMM):
                                nc.tensor.matmul(
                                    ps[:, n * NMM:(n + 1) * NMM],
                                    lhsT=lhsT,
                                    rhs=wres[:, kj, n * NMM:(n + 1) * NMM],
                                    start=(kj == 0),
                                    stop=(kj == KT - 1),
                                )
                        outt = out_pool.tile([P, N], F32, name="outt")
                        nc.vector.tensor_add(outt[:], ps[:], b_bcast[:])
                        nc.sync.dma_start(y[m * P:(m + 1) * P, :], outt[:])
    nc.compile()
    return nc


def build_bass_kv2(SCHED=(2, 2, 4, 8, 8, 8), MWIN=512, M=M, N=N, K=K,
                   num_devices=8, repeat=1, xs_bufs=3):
    """K-outer, bf16 staging + bf16 SBUF accumulator.

    Block sizes follow SCHED (sum = 32 k-tiles): small first blocks so the
    first matmuls start ~3us in instead of waiting for a full 8-k-tile W
    window; 8-k-tile blocks in steady state. wl is DMAed per 2-k-tile chunk
    (not per window) to cut first-matmul latency. yacc is bf16 (halves DVE
    evacuation cost and SBUF footprint); the final block's evacuation goes to
    f32 out tiles that are DMAed to y.
    """
    KT, MT = K // P, M // P
    assert sum(SCHED) == KT
    KB = len(SCHED)
    MGT = MWIN // P
    _patch_act_tables()
    nc = bacc.Bacc(trn_type="TRN2", target_bir_lowering=False, debug=False,
                   num_devices=num_devices)
    xt = nc.dram_tensor("xt", [K, M], BF16, kind="ExternalInput").ap()
    wl = nc.dram_tensor("wl", [K, N], BF16, kind="ExternalInput").ap()
    ws = nc.dram_tensor("ws", [K, N], BF16, kind="ExternalInput").ap()
    we = nc.dram_tensor("we", [K, N], BF16, kind="ExternalInput").ap()
    bl = nc.dram_tensor("bl", [1, N], F32, kind="ExternalInput").ap()
    bs = nc.dram_tensor("bs", [1, N], F32, kind="ExternalInput").ap()
    be = nc.dram_tensor("be", [1, N], F32, kind="ExternalInput").ap()
    y = nc.dram_tensor("y", [M, N], F32, kind="ExternalOutput").ap()
    xt_r = xt.rearrange("(kt p) m -> p kt m", p=P)
    wl_r = wl.rearrange("(kt p) n -> p kt n", p=P)
    ws_r = ws.rearrange("(kt p) n -> p kt n", p=P)
    we_r = we.rearrange("(kt p) n -> p kt n", p=P)

    from contextlib import ExitStack
    with tile.TileContext(nc) as tc, ExitStack() as rep_ctx:
        with tc.tile_pool(name="const", bufs=1) as const_pool:
            bias_state = {}

            def emit_bias():
                # inline bias prep from const-pool tiles: opening a fresh
                # tile pool mid-emission would insert a pool-close sync
                b_bcast = const_pool.tile([P, N], F32, name="b_bcast")
                bl_t = const_pool.tile([1, N], F32, name="bl_t")
                bs_t = const_pool.tile([1, N], F32, name="bs_t")
                be_t = const_pool.tile([1, N], F32, name="be_t")
                nc.scalar.dma_start(bl_t[:, :], bl[:, :])
                nc.scalar.dma_start(bs_t[:, :], bs[:, :])
                nc.scalar.dma_start(be_t[:, :], be[:, :])
                nc.scalar.activation(bs_t[:, :], bs_t[:, :], ACT.Exp)
                nc.scalar.activation(bs_t[:, :], bs_t[:, :], ACT.Ln, bias=1.0)
                nc.vector.tensor_mul(bs_t[:, :], bs_t[:, :], be_t[:, :])
                nc.vector.tensor_add(bl_t[:, :], bl_t[:, :], bs_t[:, :])
                nc.gpsimd.partition_broadcast(b_bcast[:, :], bl_t[:, :])
                bias_state["b"] = b_bcast

            with tc.tile_pool(name="yacc_pool", bufs=1) as yacc_pool, \
                 tc.tile_pool(name="wwin_pool", bufs=2) as wwin_pool, \
                 tc.tile_pool(name="wstage", bufs=2) as wstage_pool, \
                 tc.tile_pool(name="xs_pool", bufs=xs_bufs) as xs_pool, \
                 tc.tile_pool(name="out_pool", bufs=3) as out_pool, \
                 tc.tile_pool(name="psum_pool", bufs=4, space="PSUM") as psum_pool:
                if repeat > 1:
                    rep_ctx.enter_context(tc.For_i(0, repeat, 1))
                yacc = yacc_pool.tile([P, MT, N], BF16, name="yacc")

                k0 = 0
                KGMAX = max(SCHED)

                def stage_w(kb, KG, k0):
                    # one tile per 2-k-tile chunk: the dependency tracker is
                    # tile-granular, so a single [P, KG, N] window tile would
                    # make every matmul wait for the LAST staging write
                    chunks = []
                    sub = 2 if kb == 0 else 1  # 1-kt staging steps up front
                    for ci in range(KG // 2):
                        wch_t = wwin_pool.tile([P, 2, N], BF16,
                                               name=f"wch{ci}")
                        for s in range(sub):
                            h = 2 * ci + s * (2 // sub)
                            hc = 2 // sub
                            wch = wch_t[:, s * hc:(s + 1) * hc, :]
                            wsb_t = wstage_pool.tile([P, 2, N], BF16,
                                                     name="wsb_t")
                            web_t = wstage_pool.tile([P, 2, N], BF16,
                                                     name="web_t")
                            spf_t = wstage_pool.tile([P, 2, N], F32,
                                                     name="spf_t")
                            wsb, web, spf = (wsb_t[:, :hc, :],
                                             web_t[:, :hc, :],
                                             spf_t[:, :hc, :])
                            nc.sync.dma_start(wch,
                                              wl_r[:, k0 + h:k0 + h + hc, :])
                            nc.scalar.dma_start(
                                wsb, ws_r[:, k0 + h:k0 + h + hc, :])
                            nc.sync.dma_start(
                                web, we_r[:, k0 + h:k0 + h + hc, :])
                            nc.scalar.activation(spf, wsb, ACT.Exp)
                            nc.scalar.activation(spf, spf, ACT.Ln, bias=1.0)
                            nc.vector.tensor_mul(spf, spf, web)
                            nc.vector.tensor_add(wch, wch, spf)
                        chunks.append(wch_t)
                        if kb == 0 and ci == 0 and "b" not in bias_state:
                            emit_bias()  # after chunk 0: off the critical
                            # path to the first matmuls, ready before the
                            # first evacuation needs it
                    return chunks

                def load_xs(KG, k0, m0):
                    xs_t = xs_pool.tile([P, KGMAX, MWIN], BF16, name="xs")
                    xs = xs_t[:, :KG, :]
                    nc.scalar.dma_start(
                        xs[:, :, :], xt_r[:, k0:k0 + KG, m0 * P:m0 * P + MWIN])
                    return xs

                def mm(ps, xs, wchunks, kj, mi, KG, n, nn=1):
                    nc.tensor.matmul(
                        ps[:, n * NMM:(n + nn) * NMM],
                        lhsT=xs[:, kj, mi * P:(mi + 1) * P],
                        rhs=wchunks[kj // 2][:, kj % 2, n * NMM:(n + nn) * NMM],
                        start=(kj == 0),
                        stop=(kj == KG - 1),
                    )

                NN = N // NMM
                for kb, KG in enumerate(SCHED):
                    wwin = stage_w(kb, KG, k0)
                    if kb == 0:
                        # first x window: chunk-major interleave across its
                        # MGT strips so the in-order PE queue never waits on
                        # a later W chunk to run an earlier strip
                        xs = load_xs(KG, k0, 0)
                        pss = [psum_pool.tile([P, N], F32, name="ps")
                               for _ in range(MGT)]
                        for kj in range(KG):
                            for si in range(MGT):
                                for n in range(NN):
                                    mm(pss[si], xs, wwin, kj, si, KG, n)
                        for si in range(MGT):
                            nc.vector.scalar_tensor_tensor(
                                yacc[:, si, :], pss[si][:], 0.0,
                                bias_state["b"][:],
                                op0=mybir.AluOpType.add,
                                op1=mybir.AluOpType.add)
                        mg_range = range(1, MT // MGT)
                    else:
                        mg_range = range(MT // MGT)
                    for mg in mg_range:
                        m0 = mg * MGT
                        xs = load_xs(KG, k0, m0)
                        for mi in range(MGT):
                            m = m0 + mi
                            ps = psum_pool.tile([P, N], F32, name="ps")
                            last_strip = (kb == KB - 1 and m == MT - 1)
                            if last_strip:
                                # n-major with per-half evacuation: shortens
                                # the final evac+DMA tail after the last MM
                                for n in range(NN):
                                    for kj in range(KG):
                                        mm(ps, xs, wwin, kj, mi, KG, n)
                                    sl = slice(n * NMM, (n + 1) * NMM)
                                    outt = out_pool.tile([P, N], F32,
                                                         name="outt")
                                    nc.vector.tensor_add(
                                        outt[:, sl], ps[:, sl],
                                        yacc[:, m, sl])
                                    nc.sync.dma_start(
                                        y[m * P:(m + 1) * P, sl],
                                        outt[:, sl])
                                continue
                            for kj in range(KG):
                                for n in range(NN):
                                    mm(ps, xs, wwin, kj, mi, KG, n)
                            if kb == 0:
                                nc.vector.scalar_tensor_tensor(
                                    yacc[:, m, :], ps[:], 0.0,
                                    bias_state["b"][:],
                                    op0=mybir.AluOpType.add,
                                    op1=mybir.AluOpType.add)
                            elif kb < KB - 1:
                                nc.vector.tensor_add(yacc[:, m, :],
                                                     yacc[:, m, :], ps[:])
                            else:
                                outt = out_pool.tile([P, N], F32, name="outt")
                                nc.vector.tensor_add(outt[:], ps[:],
                                                     yacc[:, m, :])
                                nc.sync.dma_start(y[m * P:(m + 1) * P, :],
                                                  outt[:])
                    k0 += KG
    nc.compile()
    return nc



MW = M // 512                 # 8 x-windows of 512 batch cols per core


def _shard_inputs_v2(x, w_loc, w_std, b_loc, b_std, eps_w, eps_b):
    """Partition-major host layouts so every device DMA is 128 contiguous
    per-partition runs (one descriptor per partition):
      xtt[p, w, kt, c] = x.T[kt*128+p, w*512+c]     (per core slice)
      wlt[p, kt, n]    = w  [kt*128+p, n]
    """
    import ml_dtypes
    bf = ml_dtypes.bfloat16
    x = np.asarray(x, dtype=np.float32)
    w_loc = np.asarray(w_loc, dtype=np.float32)
    w_std = np.asarray(w_std, dtype=np.float32)
    eps_w = np.asarray(eps_w, dtype=np.float32)
    b_loc = np.asarray(b_loc, dtype=np.float32)
    b_std = np.asarray(b_std, dtype=np.float32)
    eps_b = np.asarray(eps_b, dtype=np.float32)

    def wmaj(a):  # [K, N'] -> [128, KT*N'] p-major bf16
        n = a.shape[1]
        return np.ascontiguousarray(
            a.reshape(KT, P, n).transpose(1, 0, 2).reshape(P, KT * n)
        ).astype(bf)

    in_maps = []
    for c in range(8):
        bsh, dsh = c // D_SHARD, c % D_SHARD
        ms, ns = bsh * M, dsh * N
        xt = x[ms:ms + M, :].T.astype(bf)              # [K, M]
        xtt = np.ascontiguousarray(
            xt.reshape(KT, P, MW, 512).transpose(1, 2, 0, 3).reshape(P, -1))
        in_maps.append({
            "xtt": xtt,
            "wlt": wmaj(w_loc[:, ns:ns + N]),
            "wst": wmaj(w_std[:, ns:ns + N]),
            "wet": wmaj(eps_w[:, ns:ns + N]),
            "bl": np.ascontiguousarray(b_loc[:, ns:ns + N]),
            "bs": np.ascontiguousarray(b_std[:, ns:ns + N]),
            "be": np.ascontiguousarray(eps_b[:, ns:ns + N]),
        })
    return in_maps


def build_bass_kv5(SCHED=(8, 8, 8, 8), M=M, N=N, K=K,
                   num_devices=8, repeat=1, xs_bufs=3):
    """kv2 + partition-major DRAM layouts (one descriptor per partition per
    DMA), per-chunk W tiles, xs windows hoisted onto the sync ring, y out on
    the scalar ring, bias emitted mid-staging, split last-strip evacuation."""
    KT, MT = K // P, M // P
    assert sum(SCHED) == KT
    KB = len(SCHED)
    MGT = 4                            # strips per 512-col x window
    _patch_act_tables()
    nc = bacc.Bacc(trn_type="TRN2", target_bir_lowering=False, debug=False,
                   num_devices=num_devices)
    xtt = nc.dram_tensor("xtt", [P, MW * KT * 512], BF16,
                         kind="ExternalInput").ap()
    wlt = nc.dram_tensor("wlt", [P, KT * N], BF16, kind="ExternalInput").ap()
    wst = nc.dram_tensor("wst", [P, KT * N], BF16, kind="ExternalInput").ap()
    wet = nc.dram_tensor("wet", [P, KT * N], BF16, kind="ExternalInput").ap()
    bl = nc.dram_tensor("bl", [1, N], F32, kind="ExternalInput").ap()
    bs = nc.dram_tensor("bs", [1, N], F32, kind="ExternalInput").ap()
    be = nc.dram_tensor("be", [1, N], F32, kind="ExternalInput").ap()
    y = nc.dram_tensor("y", [M, N], F32, kind="ExternalOutput").ap()
    xtt_r = xtt.rearrange("p (w kt c) -> p w kt c", w=MW, kt=KT, c=512)
    wlt_r = wlt.rearrange("p (kt n) -> p kt n", kt=KT)
    wst_r = wst.rearrange("p (kt n) -> p kt n", kt=KT)
    wet_r = wet.rearrange("p (kt n) -> p kt n", kt=KT)

    from contextlib import ExitStack
    with tile.TileContext(nc) as tc, ExitStack() as rep_ctx:
        with tc.tile_pool(name="const", bufs=1) as const_pool, \
             tc.tile_pool(name="yacc_pool", bufs=1) as yacc_pool, \
             tc.tile_pool(name="wwin_pool", bufs=2) as wwin_pool, \
             tc.tile_pool(name="wstage", bufs=2) as wstage_pool, \
             tc.tile_pool(name="xs_pool", bufs=xs_bufs) as xs_pool, \
             tc.tile_pool(name="out_pool", bufs=3) as out_pool, \
             tc.tile_pool(name="psum_pool", bufs=4, space="PSUM") as psum_pool:
            if repeat > 1:
                rep_ctx.enter_context(tc.For_i(0, repeat, 1))
            yacc = yacc_pool.tile([P, MT, N], BF16, name="yacc")
            bias_state = {}

            def emit_bias():
                b_bcast = const_pool.tile([P, N], F32, name="b_bcast")
                bl_t = const_pool.tile([1, N], F32, name="bl_t")
                bs_t = const_pool.tile([1, N], F32, name="bs_t")
                be_t = const_pool.tile([1, N], F32, name="be_t")
                nc.scalar.dma_start(bl_t[:, :], bl[:, :])
                nc.scalar.dma_start(bs_t[:, :], bs[:, :])
                nc.scalar.dma_start(be_t[:, :], be[:, :])
                nc.scalar.activation(bs_t[:, :], bs_t[:, :], ACT.Exp)
                nc.scalar.activation(bs_t[:, :], bs_t[:, :], ACT.Ln, bias=1.0)
                nc.vector.tensor_mul(bs_t[:, :], bs_t[:, :], be_t[:, :])
                nc.vector.tensor_add(bl_t[:, :], bl_t[:, :], bs_t[:, :])
                nc.gpsimd.partition_broadcast(b_bcast[:, :], bl_t[:, :])
                bias_state["b"] = b_bcast

            def load_xs(KG, k0, w):
                xs = xs_pool.tile([P, 8, 512], BF16, name="xs")
                nc.sync.dma_start(xs[:, :KG, :], xtt_r[:, w, k0:k0 + KG, :])
                return xs

            def stage_w(kb, KG, k0, hook=None):
                chunks = []
                sub = 2 if kb == 0 else 1   # 1-kt staging steps up front
                for ci in range(KG // 2):
                    wch_t = wwin_pool.tile([P, 2, N], BF16, name=f"wch{ci}")
                    for s in range(sub):
                        hc = 2 // sub
                        h = 2 * ci + s * hc
                        wch = wch_t[:, s * hc:(s + 1) * hc, :]
                        wsb_t = wstage_pool.tile([P, 2, N], BF16, name="wsb_t")
                        web_t = wstage_pool.tile([P, 2, N], BF16, name="web_t")
                        spf_t = wstage_pool.tile([P, 2, N], F32, name="spf_t")
                        wsb, web, spf = (wsb_t[:, :hc, :], web_t[:, :hc, :],
                                         spf_t[:, :hc, :])
                        nc.sync.dma_start(wch, wlt_r[:, k0 + h:k0 + h + hc, :])
                        nc.scalar.dma_start(wsb,
                                            wst_r[:, k0 + h:k0 + h + hc, :])
                        nc.sync.dma_start(web,
                                          wet_r[:, k0 + h:k0 + h + hc, :])
                        if hook is not None:
                            hook()      # e.g. xs0 DMA right after c0s0's
                            hook = None  # W transfers on the sync ring
                        nc.scalar.activation(spf, wsb, ACT.Exp)
                        nc.scalar.activation(spf, spf, ACT.Ln, bias=1.0)
                        nc.vector.tensor_mul(spf, spf, web)
                        nc.vector.tensor_add(wch, wch, spf)
                    chunks.append(wch_t)
                    if kb == 0 and ci == 0 and "b" not in bias_state:
                        emit_bias()
                return chunks

            def mm(ps, xs, wchunks, kj, mi, KG, n):
                nc.tensor.matmul(
                    ps[:, n * NMM:(n + 1) * NMM],
                    lhsT=xs[:, kj, mi * P:(mi + 1) * P],
                    rhs=wchunks[kj // 2][:, kj % 2, n * NMM:(n + 1) * NMM],
                    start=(kj == 0),
                    stop=(kj == KG - 1),
                )

            NN = N // NMM
            k0 = 0
            xs_first = {}
            for kb, KG in enumerate(SCHED):
                if kb == 0:
                    def _hook():
                        xs_first["t"] = load_xs(SCHED[0], 0, 0)
                    wchunks = stage_w(kb, KG, k0, hook=_hook)
                    xs0 = xs_first["t"]
                else:
                    wchunks = stage_w(kb, KG, k0)
                if kb == 0:
                    # chunk-major across the first window's strips: the
                    # in-order PE queue never waits on a later W chunk
                    pss = [psum_pool.tile([P, N], F32, name="ps")
                           for _ in range(MGT)]
                    for kj in range(KG):
                        for si in range(MGT):
                            for n in range(NN):
                                mm(pss[si], xs0, wchunks, kj, si, KG, n)
                    for si in range(MGT):
                        nc.vector.scalar_tensor_tensor(
                            yacc[:, si, :], pss[si][:], 0.0,
                            bias_state["b"][:],
                            op0=mybir.AluOpType.add, op1=mybir.AluOpType.add)
                    mg_range = range(1, MT // MGT)
                else:
                    mg_range = range(MT // MGT)
                for mg in mg_range:
                    m0 = mg * MGT
                    xs = load_xs(KG, k0, mg)
                    for mi in range(MGT):
                        m = m0 + mi
                        ps = psum_pool.tile([P, N], F32, name="ps")
                        if kb == KB - 1 and m == MT - 1:
                            for n in range(NN):   # split final evacuation
                                for kj in range(KG):
                                    mm(ps, xs, wchunks, kj, mi, KG, n)
                                sl = slice(n * NMM, (n + 1) * NMM)
                                outt = out_pool.tile([P, N], F32, name="outt")
                                nc.vector.tensor_add(outt[:, sl], ps[:, sl],
                                                     yacc[:, m, sl])
                                nc.sync.dma_start(y[m * P:(m + 1) * P, sl],
                                                  outt[:, sl])
                            continue
                        for kj in range(KG):
                            for n in range(NN):
                                mm(ps, xs, wchunks, kj, mi, KG, n)
                        if kb == 0:
                            nc.vector.scalar_tensor_tensor(
                                yacc[:, m, :], ps[:], 0.0, bias_state["b"][:],
                                op0=mybir.AluOpType.add,
                                op1=mybir.AluOpType.add)
                        elif kb < KB - 1:
                            nc.vector.tensor_add(yacc[:, m, :],
                                                 yacc[:, m, :], ps[:])
                        else:
                            outt = out_pool.tile([P, N], F32, name="outt")
                            nc.vector.tensor_add(outt[:], ps[:], yacc[:, m, :])
                            nc.scalar.dma_start(y[m * P:(m + 1) * P, :],
                                                outt[:])
                k0 += KG
    nc.compile()
    return nc

F8 = mybir.dt.float8e4          # TRN float8e4 == ml_dtypes.float8_e4m3 (max 240)
KF8 = 8                         # k-tiles computed in fp8 DoubleRow (of 32)
W8SCALE = 256.0                 # w values for fp8 k-tiles pre-scaled by this


def _shard_inputs_v3(x, w_loc, w_std, b_loc, b_std, eps_w, eps_b):
    """kv5 partition-major layouts, split for mixed precision:
      xtt8[p, w, kt, c]  e4m3   k-tiles 0..KF8   (x quantized at scale 1)
      xtt [p, w, kt, c]  bf16   k-tiles KF8..32
      wlt/wet[p, kt, n]  bf16   full 32 kt; kt<KF8 values pre-scaled x256 so
                                the staged fp8 W chunk is (w * 256) -> e4m3
      wst unscaled (softplus input).
    """
    import ml_dtypes
    bf = ml_dtypes.bfloat16
    f8 = ml_dtypes.float8_e4m3
    x = np.asarray(x, dtype=np.float32)
    w_loc = np.asarray(w_loc, dtype=np.float32)
    w_std = np.asarray(w_std, dtype=np.float32)
    eps_w = np.asarray(eps_w, dtype=np.float32)
    b_loc = np.asarray(b_loc, dtype=np.float32)
    b_std = np.asarray(b_std, dtype=np.float32)
    eps_b = np.asarray(eps_b, dtype=np.float32)
    KB16 = KT - KF8

    def wmaj(a, scale_f8=False):  # [K, N'] -> [128, KT*n] p-major bf16
        n = a.shape[1]
        if scale_f8:
            a = a.copy()
            a[:KF8 * P, :] *= W8SCALE
        return np.ascontiguousarray(
            a.reshape(KT, P, n).transpose(1, 0, 2).reshape(P, KT * n)
        ).astype(bf)

    in_maps = []
    for c in range(8):
        bsh, dsh = c // D_SHARD, c % D_SHARD
        ms, ns = bsh * M, dsh * N
        xt = x[ms:ms + M, :].T                          # [K, M] f32
        x8 = xt[:KF8 * P, :].astype(f8)                 # [KF8*128, M]
        xtt8 = np.ascontiguousarray(
            x8.reshape(KF8, P, MW, 512).transpose(1, 2, 0, 3).reshape(P, -1))
        xb = xt[KF8 * P:, :].astype(bf)                 # [KB16*128, M]
        xtt = np.ascontiguousarray(
            xb.reshape(KB16, P, MW, 512).transpose(1, 2, 0, 3).reshape(P, -1))
        in_maps.append({
            "xtt8": xtt8,
            "xtt": xtt,
            "wlt": wmaj(w_loc[:, ns:ns + N], scale_f8=True),
            "wst": wmaj(w_std[:, ns:ns + N]),
            "wet": wmaj(eps_w[:, ns:ns + N], scale_f8=True),
            "bl": np.ascontiguousarray(b_loc[:, ns:ns + N]),
            "bs": np.ascontiguousarray(b_std[:, ns:ns + N]),
            "be": np.ascontiguousarray(eps_b[:, ns:ns + N]),
        })
    return in_maps


def build_bass_kv6(SCHED=(KF8, 8, 8, 8), M=M, N=N, K=K,
                   num_devices=8, repeat=1, xs_bufs=2):
    """kv5 + first block in fp8 e4m3 DoubleRow.

    Block 0 covers KF8 k-tiles: x comes pre-quantized to e4m3 (host), W chunks
    are staged exactly like kv5 but the staging add writes an e4m3 tile whose
    values are w*256 (wlt/wet pre-scaled on host; cast saturates far below 240
    since |256*w| <~ 25). Each DR matmul contracts a 2-k-tile pair ([128,2,128]
    stationary x, [128,2,512] moving W) at ~0.56x the bf16 cycle cost; the
    block evacuation folds the 1/256 descale into its existing DVE op.
    Remaining blocks are bf16, unchanged from kv5.
    """
    KT, MT = K // P, M // P
    assert sum(SCHED) == KT and SCHED[0] == KF8 and KF8 % 2 == 0
    KB = len(SCHED)
    KGMAXB = max(SCHED[1:])            # widest bf16 block
    KB16 = KT - KF8
    MGT = 4                            # strips per 512-col x window
    NP8 = KF8 // 2                     # DR pair groups in block 0
    DR = mybir.MatmulPerfMode.DoubleRow
    _patch_act_tables()
    nc = bacc.Bacc(trn_type="TRN2", target_bir_lowering=False, debug=False,
                   num_devices=num_devices)
    xtt8 = nc.dram_tensor("xtt8", [P, MW * KF8 * 512], F8,
                          kind="ExternalInput").ap()
    xtt = nc.dram_tensor("xtt", [P, MW * KB16 * 512], BF16,
                         kind="ExternalInput").ap()
    wlt = nc.dram_tensor("wlt", [P, KT * N], BF16, kind="ExternalInput").ap()
    wst = nc.dram_tensor("wst", [P, KT * N], BF16, kind="ExternalInput").ap()
    wet = nc.dram_tensor("wet", [P, KT * N], BF16, kind="ExternalInput").ap()
    bl = nc.dram_tensor("bl", [1, N], F32, kind="ExternalInput").ap()
    bs = nc.dram_tensor("bs", [1, N], F32, kind="ExternalInput").ap()
    be = nc.dram_tensor("be", [1, N], F32, kind="ExternalInput").ap()
    y = nc.dram_tensor("y", [M, N], F32, kind="ExternalOutput").ap()
    xtt8_r = xtt8.rearrange("p (w kt c) -> p w kt c", w=MW, kt=KF8, c=512)
    xtt_r = xtt.rearrange("p (w kt c) -> p w kt c", w=MW, kt=KB16, c=512)
    wlt_r = wlt.rearrange("p (kt n) -> p kt n", kt=KT)
    wst_r = wst.rearrange("p (kt n) -> p kt n", kt=KT)
    wet_r = wet.rearrange("p (kt n) -> p kt n", kt=KT)

    from contextlib import ExitStack
    with tile.TileContext(nc) as tc, ExitStack() as rep_ctx:
        with tc.tile_pool(name="const", bufs=1) as const_pool, \
             tc.tile_pool(name="yacc_pool", bufs=1) as yacc_pool, \
             tc.tile_pool(name="wwin_pool", bufs=2) as wwin_pool, \
             tc.tile_pool(name="w8_pool", bufs=1) as w8_pool, \
             tc.tile_pool(name="wstage", bufs=2) as wstage_pool, \
             tc.tile_pool(name="xs_pool", bufs=xs_bufs) as xs_pool, \
             tc.tile_pool(name="xs8_pool", bufs=2) as xs8_pool, \
             tc.tile_pool(name="out_pool", bufs=3) as out_pool, \
             tc.tile_pool(name="psum_pool", bufs=4, space="PSUM") as psum_pool:
            # HAM warmup: ~10 dummy matmuls before the For_i body so the PE
            # clock-gate opens (4096-cycle busy window) while the first W
            # chunk is still staging; first real matmuls then run at 2.4GHz.
            # Outside the repeat loop: helps the single-shot fill only.
            wmu = const_pool.tile([P, NMM], BF16, name="wmu")
            nc.any.memset(wmu[:, :], 0)
            psw = psum_pool.tile([P, N], F32, name="ps")  # rotation slot 0
            for _ in range(10):
                nc.tensor.matmul(psw[:, :NMM], lhsT=wmu[:, 0:P],
                                 rhs=wmu[:, :], start=True, stop=True)
            if repeat > 1:
                rep_ctx.enter_context(tc.For_i(0, repeat, 1))
            yacc = yacc_pool.tile([P, MT, N], BF16, name="yacc")
            bias_state = {}

            def emit_bias():
                b_bcast = const_pool.tile([P, N], F32, name="b_bcast")
                bl_t = const_pool.tile([1, N], F32, name="bl_t")
                bs_t = const_pool.tile([1, N], F32, name="bs_t")
                be_t = const_pool.tile([1, N], F32, name="be_t")
                nc.scalar.dma_start(bl_t[:, :], bl[:, :])
                nc.scalar.dma_start(bs_t[:, :], bs[:, :])
                nc.scalar.dma_start(be_t[:, :], be[:, :])
                nc.scalar.activation(bs_t[:, :], bs_t[:, :], ACT.Exp)
                nc.scalar.activation(bs_t[:, :], bs_t[:, :], ACT.Ln, bias=1.0)
                nc.vector.tensor_mul(bs_t[:, :], bs_t[:, :], be_t[:, :])
                nc.vector.tensor_add(bl_t[:, :], bl_t[:, :], bs_t[:, :])
                nc.gpsimd.partition_broadcast(b_bcast[:, :], bl_t[:, :])
                bias_state["b"] = b_bcast

            def load_xs8(w):
                xs = xs8_pool.tile([P, KF8, 512], F8, name="xs8")
                nc.sync.dma_start(xs[:], xtt8_r[:, w, :, :])
                return xs

            def load_xs(KG, k0, w):
                xs = xs_pool.tile([P, KGMAXB, 512], BF16, name="xs")
                kb0 = k0 - KF8
                nc.sync.dma_start(xs[:, :KG, :], xtt_r[:, w, kb0:kb0 + KG, :])
                return xs

            def stage_w8(hook=None):
                """Block-0 W chunks -> e4m3 tiles holding 256*w."""
                chunks = []
                for ci in range(NP8):
                    wch8 = w8_pool.tile([P, 2, N], F8, name=f"w8c{ci}")
                    for s in range(2):      # 1-kt staging substeps
                        h = 2 * ci + s
                        wlb_t = wstage_pool.tile([P, 2, N], BF16, name="wlb_t")
                        wsb_t = wstage_pool.tile([P, 2, N], BF16, name="wsb_t")
                        web_t = wstage_pool.tile([P, 2, N], BF16, name="web_t")
                        spf_t = wstage_pool.tile([P, 2, N], BF16, name="spf_t")
                        wlb, wsb, web, spf = (wlb_t[:, :1, :], wsb_t[:, :1, :],
                                              web_t[:, :1, :], spf_t[:, :1, :])
                        nc.sync.dma_start(wlb, wlt_r[:, h:h + 1, :])
                        nc.scalar.dma_start(wsb, wst_r[:, h:h + 1, :])
                        nc.sync.dma_start(web, wet_r[:, h:h + 1, :])
                        if hook is not None:
                            hook()
                            hook = None
                        nc.scalar.activation(spf, wsb, ACT.Exp)
                        nc.scalar.activation(spf, spf, ACT.Ln, bias=1.0)
                        nc.vector.tensor_mul(spf, spf, web)
                        nc.vector.tensor_add(wch8[:, s:s + 1, :], wlb, spf)
                    chunks.append(wch8)
                    if ci == 0 and "b" not in bias_state:
                        emit_bias()
                return chunks

            def stage_w(KG, k0):
                chunks = []
                for ci in range(KG // 2):
                    wch_t = wwin_pool.tile([P, 2, N], BF16, name=f"wch{ci}")
                    wsb_t = wstage_pool.tile([P, 2, N], BF16, name="wsb_t")
                    web_t = wstage_pool.tile([P, 2, N], BF16, name="web_t")
                    spf_t = wstage_pool.tile([P, 2, N], BF16, name="spf_t")
                    h = 2 * ci
                    nc.sync.dma_start(wch_t[:], wlt_r[:, k0 + h:k0 + h + 2, :])
                    nc.scalar.dma_start(wsb_t[:], wst_r[:, k0 + h:k0 + h + 2, :])
                    nc.sync.dma_start(web_t[:], wet_r[:, k0 + h:k0 + h + 2, :])
                    nc.scalar.activation(spf_t[:], wsb_t[:], ACT.Exp)
                    nc.scalar.activation(spf_t[:], spf_t[:], ACT.Ln, bias=1.0)
                    nc.vector.tensor_mul(spf_t[:], spf_t[:], web_t[:])
                    nc.vector.tensor_add(wch_t[:], wch_t[:], spf_t[:])
                    chunks.append(wch_t)
                return chunks

            def mm(ps, xs, wchunks, kj, mi, KG, n):
                nc.tensor.matmul(
                    ps[:, n * NMM:(n + 1) * NMM],
                    lhsT=xs[:, kj, mi * P:(mi + 1) * P],
                    rhs=wchunks[kj // 2][:, kj % 2, n * NMM:(n + 1) * NMM],
                    start=(kj == 0),
                    stop=(kj == KG - 1),
                )

            def mm8(ps, xs8, wchunks8, j, mi, n):
                nc.tensor.matmul(
                    ps[:, n * NMM:(n + 1) * NMM],
                    lhsT=xs8[:, 2 * j:2 * j + 2, mi * P:(mi + 1) * P],
                    rhs=wchunks8[j][:, :, n * NMM:(n + 1) * NMM],
                    start=(j == 0),
                    stop=(j == NP8 - 1),
                    perf_mode=DR,
                )

            NN = N // NMM
            k0 = 0
            xs_first = {}
            for kb, KG in enumerate(SCHED):
                if kb == 0:
                    def _hook():
                        xs_first["t"] = load_xs8(0)
                    wch8s = stage_w8(hook=_hook)
                    xs0 = xs_first["t"]
                    # chunk-major across the first window's strips
                    pss = [psum_pool.tile([P, N], F32, name="ps")
                           for _ in range(MGT)]
                    for j in range(NP8):
                        for si in range(MGT):
                            for n in range(NN):
                                mm8(pss[si], xs0, wch8s, j, si, n)
                    for si in range(MGT):
                        nc.vector.scalar_tensor_tensor(
                            yacc[:, si, :], pss[si][:], 1.0 / W8SCALE,
                            bias_state["b"][:],
                            op0=mybir.AluOpType.mult, op1=mybir.AluOpType.add)
                    for mg in range(1, MT // MGT):
                        xs8 = load_xs8(mg)
                        for mi in range(MGT):
                            m = mg * MGT + mi
                            ps = psum_pool.tile([P, N], F32, name="ps")
                            for j in range(NP8):
                                for n in range(NN):
                                    mm8(ps, xs8, wch8s, j, mi, n)
                            nc.vector.scalar_tensor_tensor(
                                yacc[:, m, :], ps[:], 1.0 / W8SCALE,
                                bias_state["b"][:],
                                op0=mybir.AluOpType.mult,
                                op1=mybir.AluOpType.add)
                    k0 += KG
                    continue
                wchunks = stage_w(KG, k0)
                for mg in range(MT // MGT):
                    m0 = mg * MGT
                    xs = load_xs(KG, k0, mg)
                    for mi in range(MGT):
                        m = m0 + mi
                        ps = psum_pool.tile([P, N], F32, name="ps")
                        if kb == KB - 1 and m == MT - 1:
                            for n in range(NN):   # split final evacuation
                                for kj in range(KG):
                                    mm(ps, xs, wchunks, kj, mi, KG, n)
                                sl = slice(n * NMM, (n + 1) * NMM)
                                outt = out_pool.tile([P, N], F32, name="outt")
                                nc.vector.tensor_add(outt[:, sl], ps[:, sl],
                                                     yacc[:, m, sl])
                                nc.sync.dma_start(y[m * P:(m + 1) * P, sl],
                                                  outt[:, sl])
                            continue
                        for kj in range(KG):
                            for n in range(NN):
                                mm(ps, xs, wchunks, kj, mi, KG, n)
                        if kb < KB - 1:
                            nc.vector.tensor_add(yacc[:, m, :],
                                                 yacc[:, m, :], ps[:])
                        else:
                            outt = out_pool.tile([P, N], F32, name="outt")
                            nc.vector.tensor_add(outt[:], ps[:], yacc[:, m, :])
                            nc.scalar.dma_start(y[m * P:(m + 1) * P, :],
                                                outt[:])
                k0 += KG
    nc.compile()
    return nc


# Which kernel build kernel() ships with: "b16" (bf16-staged inputs, ~2x less
# DMA) or "f32r" (full fp32 staging, ~25x lower error, ~15% slower) or "mres"
# (bf16 + W fully SBUF-resident, full-K PSUM accumulation) or "kv2"
# (k-outer bf16 with scheduled block sizes + bf16 accumulator) or "kv6"
# (kv5 + first 6 k-tiles in fp8 e4m3 DoubleRow).
VARIANT = "kv6"

_BUILDERS = {
    "f32r": build_bass_kouter,
    "b16": build_bass_kouter_b16,
    "mres": build_bass_mres,
    "kv2": build_bass_kv2,
    "kv5": build_bass_kv5,
    "kv6": build_bass_kv6,
}


def shard_inputs(**inputs):
    if VARIANT == "kv6":
        return _shard_inputs_v3(**inputs)
    if VARIANT == "kv5":
        return _shard_inputs_v2(**inputs)
    return _shard_inputs(**inputs, b16=(VARIANT != "f32r"))


def _get_nc():
    if "nc" not in _CACHE:
        _CACHE["nc"] = _BUILDERS[VARIANT]()
    return _CACHE["nc"]


def _shard_inputs(x, w_loc, w_std, b_loc, b_std, eps_w, eps_b, b16=False):
    import ml_dtypes
    wdt = ml_dtypes.bfloat16 if b16 else np.float32
    xt_full = np.asarray(x, dtype=np.float32).T.astype(wdt)  # [K, BATCH]
    w_loc = np.asarray(w_loc, dtype=np.float32).astype(wdt)
    w_std = np.asarray(w_std, dtype=np.float32).astype(wdt)
    eps_w = np.asarray(eps_w, dtype=np.float32).astype(wdt)
    b_loc = np.asarray(b_loc, dtype=np.float32)
    b_std = np.asarray(b_std, dtype=np.float32)
    eps_b = np.asarray(eps_b, dtype=np.float32)

    in_maps = []
    for c in range(8):
        bsh, dsh = c // D_SHARD, c % D_SHARD
        ms, ns = bsh * M, dsh * N
        in_maps.append({
            "xt": np.ascontiguousarray(xt_full[:, ms:ms + M]),
            "wl": np.ascontiguousarray(w_loc[:, ns:ns + N]),
            "ws": np.ascontiguousarray(w_std[:, ns:ns + N]),
            "we": np.ascontiguousarray(eps_w[:, ns:ns + N]),
            "bl": np.ascontiguousarray(b_loc[:, ns:ns + N]),
            "bs": np.ascontiguousarray(b_std[:, ns:ns + N]),
            "be": np.ascontiguousarray(eps_b[:, ns:ns + N]),
        })
    return in_maps


def run_profiled(inputs, trace=False, **kwargs):
    """Returns (full_output [8192,4096] f32, BassKernelResults)."""
    nc = _get_nc()
    in_maps = shard_inputs(**inputs)
    res = run_bass_kernel_spmd(nc, in_maps, core_ids=list(range(8)), trace=trace,
                               **kwargs)
    out = np.empty((BATCH, D_OUT), dtype=np.float32)
    for c in range(8):
        bsh, dsh = c // D_SHARD, c % D_SHARD
        out[bsh * M:(bsh + 1) * M, dsh * N:(dsh + 1) * N] = res.results[c]["y"]
    return out, res


def kernel(**inputs) -> np.ndarray:
    out, _ = run_profiled(inputs, trace=False)
    return out


def build_pe_probe(KG=8, M=M, N=N, K=K, num_devices=8, repeat=1, evac=False):
    """Diagnostic: the same 2048-matmul PE stream as kv2 but with W and x
    staged once (tiny DMA) and no per-strip evacuation (unless evac=True) —
    isolates the HW per-matmul cost incl. issue/LDWEIGHTS/HAM effects."""
    KT, MT = K // P, M // P
    KB = KT // KG
    nc = bacc.Bacc(trn_type="TRN2", target_bir_lowering=False, debug=False,
                   num_devices=num_devices)
    wl = nc.dram_tensor("wl", [K, N], BF16, kind="ExternalInput").ap()
    xt = nc.dram_tensor("xt", [K, M], BF16, kind="ExternalInput").ap()
    y = nc.dram_tensor("y", [M, N], F32, kind="ExternalOutput").ap()
    wl_r = wl.rearrange("(kt p) n -> p kt n", p=P)
    xt_r = xt.rearrange("(kt p) m -> p kt m", p=P)

    from contextlib import ExitStack
    with tile.TileContext(nc) as tc, ExitStack() as rep_ctx:
        with tc.tile_pool(name="wwin_pool", bufs=1) as wwin_pool, \
             tc.tile_pool(name="xs_pool", bufs=1) as xs_pool, \
             tc.tile_pool(name="out_pool", bufs=2) as out_pool, \
             tc.tile_pool(name="psum_pool", bufs=4, space="PSUM") as psum_pool:
            wwin = wwin_pool.tile([P, KG, N], BF16, name="wwin")
            xs = xs_pool.tile([P, KG, P], BF16, name="xs")
            nc.sync.dma_start(wwin[:], wl_r[:, 0:KG, :])
            nc.scalar.dma_start(xs[:], xt_r[:, 0:KG, 0:P])
            if repeat > 1:
                rep_ctx.enter_context(tc.For_i(0, repeat, 1))
            for kb in range(KB):
                for m in range(MT):
                    ps = psum_pool.tile([P, N], F32, name="ps")
                    for kj in range(KG):
                        lhsT = xs[:, kj, :]
                        for n in range(N // NMM):
                            nc.tensor.matmul(
                                ps[:, n * NMM:(n + 1) * NMM],
                                lhsT=lhsT,
                                rhs=wwin[:, kj, n * NMM:(n + 1) * NMM],
                                start=(kj == 0),
                                stop=(kj == KG - 1),
                            )
                    if evac or (kb == KB - 1 and m == MT - 1):
                        outt = out_pool.tile([P, N], F32, name="outt")
                        nc.vector.tensor_copy(outt[:], ps[:])
                        if kb == KB - 1 and m == MT - 1:
                            nc.sync.dma_start(y[0:P, :], outt[:])
    nc.compile()
    return nc


def build_pe_probe2(KG=8, M=M, N=N, K=K, num_devices=8, repeat=1):
    """Probe: same 2048-matmul stream but 4 MMs per LDWEIGHTS (each n-half
    written twice) over 16 strips — if sustained ns/MM drops vs build_pe_probe,
    LDWEIGHTS is not being hidden behind matmul streaming."""
    KT = K // P
    KB = KT // KG
    nc = bacc.Bacc(trn_type="TRN2", target_bir_lowering=False, debug=False,
                   num_devices=num_devices)
    wl = nc.dram_tensor("wl", [K, N], BF16, kind="ExternalInput").ap()
    xt = nc.dram_tensor("xt", [K, M], BF16, kind="ExternalInput").ap()
    y = nc.dram_tensor("y", [M, N], F32, kind="ExternalOutput").ap()
    wl_r = wl.rearrange("(kt p) n -> p kt n", p=P)
    xt_r = xt.rearrange("(kt p) m -> p kt m", p=P)

    from contextlib import ExitStack
    with tile.TileContext(nc) as tc, ExitStack() as rep_ctx:
        with tc.tile_pool(name="wwin_pool", bufs=1) as wwin_pool, \
             tc.tile_pool(name="xs_pool", bufs=1) as xs_pool, \
             tc.tile_pool(name="out_pool", bufs=2) as out_pool, \
             tc.tile_pool(name="psum_pool", bufs=4, space="PSUM") as psum_pool:
            wwin = wwin_pool.tile([P, KG, N], BF16, name="wwin")
            xs = xs_pool.tile([P, KG, P], BF16, name="xs")
            nc.sync.dma_start(wwin[:], wl_r[:, 0:KG, :])
            nc.scalar.dma_start(xs[:], xt_r[:, 0:KG, 0:P])
            if repeat > 1:
                rep_ctx.enter_context(tc.For_i(0, repeat, 1))
            for kb in range(KB):
                for m in range(16):
                    ps = psum_pool.tile([P, N], F32, name="ps")
                    for kj in range(KG):
                        lhsT = xs[:, kj, :]
                        for rep2 in range(2):
                            for n in range(N // NMM):
                                nc.tensor.matmul(
                                    ps[:, n * NMM:(n + 1) * NMM],
                                    lhsT=lhsT,
                                    rhs=wwin[:, kj, n * NMM:(n + 1) * NMM],
                                    start=(kj == 0 and rep2 == 0),
                                    stop=(kj == KG - 1 and rep2 == 1),
                                )
                    if kb == KB - 1 and m == 15:
                        outt = out_pool.tile([P, N], F32, name="outt")
                        nc.vector.tensor_copy(outt[:], ps[:])
                        nc.sync.dma_start(y[0:P, :], outt[:])
    nc.compile()
    return nc


def build_pe_probe4(M=M, N=N, K=K, num_devices=8, repeat=1):
    """Pure-stream probe: ONE LDWEIGHTS ever, 2048 independent 512-col bf16
    matmuls (start=stop=True) into rotating psum tiles with no readers —
    measures the max sustainable MM rate (clock cap vs HAM oscillation)."""
    nc = bacc.Bacc(trn_type="TRN2", target_bir_lowering=False, debug=False,
                   num_devices=num_devices)
    wl = nc.dram_tensor("wl", [K, N], BF16, kind="ExternalInput").ap()
    xt = nc.dram_tensor("xt", [K, M], BF16, kind="ExternalInput").ap()
    y = nc.dram_tensor("y", [P, N], F32, kind="ExternalOutput").ap()
    wl_r = wl.rearrange("(kt p) n -> p kt n", p=P)
    xt_r = xt.rearrange("(kt p) m -> p kt m", p=P)

    from contextlib import ExitStack
    with tile.TileContext(nc) as tc, ExitStack() as rep_ctx:
        with tc.tile_pool(name="wwin_pool", bufs=1) as wwin_pool, \
             tc.tile_pool(name="xs_pool", bufs=1) as xs_pool, \
             tc.tile_pool(name="out_pool", bufs=1) as out_pool, \
             tc.tile_pool(name="psum_pool", bufs=4, space="PSUM") as psum_pool:
            wwin = wwin_pool.tile([P, N], BF16, name="wwin")
            xs = xs_pool.tile([P, P], BF16, name="xs")
            nc.sync.dma_start(wwin[:], wl_r[:, 0, :])
            nc.scalar.dma_start(xs[:], xt_r[:, 0, 0:P])
            if repeat > 1:
                rep_ctx.enter_context(tc.For_i(0, repeat, 1))
            lhsT = xs[:, :]
            ps = None
            for g in range(512):
                ps = psum_pool.tile([P, N], F32, name="ps")
                for n in range(4):
                    nc.tensor.matmul(
                        ps[:, (n % 2) * NMM:(n % 2 + 1) * NMM],
                        lhsT=lhsT,
                        rhs=wwin[:, (n % 2) * NMM:(n % 2 + 1) * NMM],
                        start=True, stop=True,
                    )
            outt = out_pool.tile([P, N], F32, name="outt")
            nc.vector.tensor_copy(outt[:], ps[:])
            nc.sync.dma_start(y[0:P, :], outt[:])
    nc.compile()
    return nc

